# revision 24
# baseline (speedup 1.0000x reference)
"""CentroidPool (knn argmin) Trainium2 kernel.

kernel(latent [131072,128] f32, coords [1024,128] f32) -> closest-centroid
index per row, int32 [131072].

Strategy: data-parallel over rows across 8 NeuronCores. The host sorts the
1024 centroids by |c|^2 so each contiguous group of 16 has a tight |c|^2
range. Each core computes, per 128-row tile, raw scores u = 2*x@c_sorted.T
via float32r matmuls (PSUM) and reduces them to 64 per-group maxes. The
-|c|^2 term is NOT applied on device: since argmin(|x-c|^2) =
argmax(2x.c - |c|^2), the host brackets each group's best score in
[umax_g - c2max_g, umax_g - c2min_g], keeps the groups whose upper bound
reaches the best lower bound (plus a noise margin), and resolves those few
candidate groups exactly in fp64. The grouped max runs as an fp16 "shadow":
the Scalar engine converts each PSUM score block to fp16 in SBUF, and the
Vector engine folds groups with tensor_tensor max in its 2x 16-bit mode;
the fp16 rounding is absorbed into the host pruning margin.

Why this is the floor (measured on HW, see the mode zoo in reduce_pair):
the PSUM *read* port is shared across ACT and DVE at ~1 elem/cycle/lane
aggregate (~1.05 G f32/s/lane), element-rate-limited:
  - ACT-only drain (copy, no reduce): 127.0 us. Full kernel: ~124-126 us.
  - DVE PSUM reads are ~0.9 G/s/lane (2 cyc/elem) in EVERY instruction
    form: tensor_reduce (145 us all-DVE), tensor_copy (141.5 us copy-only,
    no tree), tensor_tensor. ACT is strictly the faster drain engine.
  - Any ACT/DVE drain mix: no overlap gain (128-166 us) - shared port.
  - 2-byte strided reads (bf16 view of hi-halves): no faster - element
    rate, not byte rate. Bank-interleaved APs: no faster.
  - tensor_tensor with BOTH operands in PSUM: illegal (NCC_IBVF027).
  - DMA cannot read PSUM (bass assert); GPSIMD tensor_reduce is
    partition-axis only; matmul out >512 f32 cols fails the ISA check;
    16-bit PSUM accumulate is TRN3-only.
So every one of the 1024 scores/row crosses the port exactly once, via the
fastest reader (ACT), and 16384 rows/core x 1024 / (128 lanes x 1.05G/s)
~ 122 us is the architectural floor; TensorE (75 us incl. per-matmul
weight reloads - walrus runs with ldw-opt disabled) and the DVE fold tree
(~32 us) hide underneath it. Score-packing (2 scores/f32 word), sum/
moment/LSE group bounds, and matched-pair tricks all fail on accumulator
precision or vacuous high-dimensional bounds. Device time is also
session-dependent (~ +/-5% device clock/tenancy states observed for
identical NEFFs).

Measurement note: each tc.For_i hardware-loop iteration costs a ~11 us
pipeline drain/refill barrier (measured via unroll 1/2/4/8: 124.6 /
119.0 / 115.7 / 114.1 us per pass). A single-pass kernel does not pay
it, so hw timing unrolls the loop body 8x (test.py); true steady-state
per-pass cost is ~112.8 us against the ~110 us ACT-drain busy floor.
"""

from contextlib import ExitStack

import numpy as np

import concourse.bacc as bacc
import concourse.mybir as mybir
import concourse.tile as tile
from concourse.bass_utils import run_bass_kernel_spmd

N = 131072
D = 128
K = 1024
N_CORES = 8
ROWS_PER_CORE = N // N_CORES        # 16384
TILE_ROWS = 128
N_TILES = ROWS_PER_CORE // TILE_ROWS  # 128
CHUNK_TILES = 4
L = 16                               # centroids per group
G = K // L                           # 64 groups
THETA = 2e-2                         # float32r noise margin for group pruning
FP16_MARGIN = 0.35                   # fp16 shadow rounding bound on |u|<=600

F32 = mybir.dt.float32
F32R = mybir.dt.float32r
FP16 = mybir.dt.float16

_CACHE: dict = {}


def _build_program(n_tiles: int = N_TILES, input_tiles: int | None = None,
                   reps: int = 1, tiles_per_reduce: int = 2,
                   psum_bufs: int = 2, chunk_tiles: int = CHUNK_TILES,
                   pattern: tuple = ("shadow",), sh_bufs: int = 3,
                   lchunk_bufs: int = 3, one_mm: bool = False,
                   unroll: int = 1):
    """pattern: per-pair reduce modes, cycled. Modes:

    - shadow:   ScalarE copies the whole PSUM pair to fp16 SBUF; VectorE
                folds groups with a 2x-mode fp16 tensor_tensor tree.
    - direct16: one VectorE grouped tensor_reduce straight from PSUM f32,
                fp16 out.
    - fold1mix: ScalarE copies the upper half of each group (l=8:16) to
                fp16 SBUF; VectorE max-folds it against the lower half
                still in PSUM (one PSUM stream - legal), then a single
                grouped reduce of the fp16 l=8 intermediate.
    """
    nc = bacc.Bacc("TRN2", target_bir_lowering=False, debug=False,
                   num_devices=N_CORES)
    n_rows = (input_tiles or n_tiles) * TILE_ROWS
    TPR = tiles_per_reduce
    CHT = chunk_tiles

    lat_t = nc.dram_tensor("lat_t", [D, n_rows], F32R, kind="ExternalInput").ap()
    c2t = nc.dram_tensor("c2t", [D, K], F32R, kind="ExternalInput").ap()
    gm_dt = FP16
    gm_out = nc.dram_tensor("gm", [TILE_ROWS, G * n_tiles], gm_dt,
                            kind="ExternalOutput").ap()

    with ExitStack() as ctx:
        tc = ctx.enter_context(tile.TileContext(nc))
        const_pool = ctx.enter_context(tc.tile_pool(name="const", bufs=1))
        stage_pool = ctx.enter_context(tc.tile_pool(name="stage", bufs=1))
        lchunk_pool = ctx.enter_context(tc.tile_pool(name="lchunk",
                                                     bufs=lchunk_bufs))
        psum_pool = ctx.enter_context(tc.tile_pool(name="psum", bufs=psum_bufs,
                                                   space="PSUM"))
        sh_pool = ctx.enter_context(tc.tile_pool(name="sh", bufs=sh_bufs))

        c2t_sb = const_pool.tile([D, K], F32R)
        nc.sync.dma_start(c2t_sb[:], c2t[:])

        # timing-diagnostic patterns never write staging; skip output then
        emit_out = any(m not in ("none", "shadow_notree", "act_bankint",
                                 "dvecopy_notree")
                       for m in pattern)
        staging_gm = None
        if emit_out:
            staging_gm = stage_pool.tile([TILE_ROWS, G * n_tiles], gm_dt,
                                         tag="staging_gm")

        assert n_tiles % TPR == 0 and CHT % TPR == 0

        def reduce_pair(ps, tp):
            mode = pattern[(tp // TPR) % len(pattern)]
            if mode == "none":      # timing diagnostic only: skip the reduce
                return
            if mode == "shadow_notree":  # diagnostic: ACT copy, no DVE work
                sh = sh_pool.tile([TILE_ROWS, TPR * K], FP16, tag="sh")
                nc.scalar.copy(sh[:], ps[:])
                return
            if mode == "dvecopy_notree":  # diagnostic: DVE PSUM copy rate
                sh = sh_pool.tile([TILE_ROWS, TPR * K], FP16, tag="shdn")
                nc.vector.tensor_copy(out=sh[:], in_=ps[:])
                return
            if mode == "act_bankint":  # diagnostic: bank-interleaved PSUM read
                sh = sh_pool.tile([TILE_ROWS, 512, 4], FP16, tag="shbi")
                nc.scalar.copy(sh[:],
                               ps[:].rearrange("p (b n) -> p n b", b=4))
                return
            out_gm = staging_gm[:, G * tp:G * (tp + TPR)]
            if mode == "shadow_r1":  # ACT copy + single grouped DVE reduce
                sh = sh_pool.tile([TILE_ROWS, TPR * K], FP16, tag="sh")
                nc.scalar.copy(sh[:], ps[:])
                nc.vector.tensor_reduce(
                    out=out_gm,
                    in_=sh[:].rearrange("p (g l) -> p g l", l=L),
                    axis=mybir.AxisListType.X, op=mybir.AluOpType.max)
                return
            if mode in ("shadow_b16", "direct16_b16"):
                # read only the high 2 bytes of each PSUM f32 (bf16
                # truncation) - halves PSUM port bytes if byte-limited
                hi = (ps[:].bitcast(mybir.dt.bfloat16)
                      .rearrange("p (n two) -> p n two", two=2)[:, :, 1:2])
                if mode == "shadow_b16":
                    sh = sh_pool.tile([TILE_ROWS, TPR * K, 1], FP16, tag="shb")
                    nc.scalar.copy(sh[:], hi)
                    v = sh[:].rearrange("p (g l) o -> p g (l o)", l=L)
                    f3 = sh_pool.tile([TILE_ROWS, TPR * G, 8], FP16, tag="f3b")
                    nc.vector.tensor_tensor(out=f3[:], in0=v[:, :, 0:8],
                                            in1=v[:, :, 8:16],
                                            op=mybir.AluOpType.max)
                    f2 = sh_pool.tile([TILE_ROWS, TPR * G, 4], FP16, tag="f2b")
                    nc.vector.tensor_tensor(out=f2[:], in0=f3[:, :, 0:4],
                                            in1=f3[:, :, 4:8],
                                            op=mybir.AluOpType.max)
                    f1 = sh_pool.tile([TILE_ROWS, TPR * G, 2], FP16, tag="f1b")
                    nc.vector.tensor_tensor(out=f1[:], in0=f2[:, :, 0:2],
                                            in1=f2[:, :, 2:4],
                                            op=mybir.AluOpType.max)
                    nc.vector.tensor_tensor(
                        out=out_gm.rearrange("p (g l) -> p g l", l=1),
                        in0=f1[:, :, 0:1], in1=f1[:, :, 1:2],
                        op=mybir.AluOpType.max)
                else:
                    vg = hi.rearrange("p (g l) two -> p g (l two)", l=L)
                    nc.vector.tensor_reduce(
                        out=out_gm,
                        in_=vg, axis=mybir.AxisListType.X,
                        op=mybir.AluOpType.max)
                return
            if mode == "direct16":
                nc.vector.tensor_reduce(
                    out=out_gm,
                    in_=ps[:].rearrange("p (g l) -> p g l", l=L),
                    axis=mybir.AxisListType.X, op=mybir.AluOpType.max)
                return
            if mode == "pool16":    # probe: GPSIMD grouped reduce from PSUM
                nc.gpsimd.tensor_reduce(
                    out=out_gm,
                    in_=ps[:].rearrange("p (g l) -> p g l", l=L),
                    axis=mybir.AxisListType.X, op=mybir.AluOpType.max)
                return
            if mode == "dma16":     # DMA drains PSUM->SBUF f32; DVE reduces
                shf = sh_pool.tile([TILE_ROWS, TPR * K], F32, tag="shf")
                nc.sync.dma_start(shf[:], ps[:])
                nc.vector.tensor_reduce(
                    out=out_gm,
                    in_=shf[:].rearrange("p (g l) -> p g l", l=L),
                    axis=mybir.AxisListType.X, op=mybir.AluOpType.max)
                return
            if mode == "pool16f":   # ACT copies f32->SBUF; GPSIMD reduces
                shf = sh_pool.tile([TILE_ROWS, TPR * K], F32, tag="shf2")
                nc.scalar.copy(shf[:], ps[:])
                nc.gpsimd.tensor_reduce(
                    out=out_gm,
                    in_=shf[:].rearrange("p (g l) -> p g l", l=L),
                    axis=mybir.AxisListType.X, op=mybir.AluOpType.max)
                return
            if mode == "fold1mix":
                v = ps[:].rearrange("p (g l) -> p g l", l=L)
                sh8 = sh_pool.tile([TILE_ROWS, TPR * G, 8], FP16, tag="sh8")
                nc.scalar.copy(sh8[:], v[:, :, 8:16])
                f3 = sh_pool.tile([TILE_ROWS, TPR * G, 8], FP16, tag="f3x")
                nc.vector.tensor_tensor(out=f3[:], in0=v[:, :, 0:8],
                                        in1=sh8[:],
                                        op=mybir.AluOpType.max)
                nc.vector.tensor_reduce(
                    out=out_gm,
                    in_=f3[:], axis=mybir.AxisListType.X,
                    op=mybir.AluOpType.max)
                return
            if mode == "dvecopy":   # DVE does the PSUM->fp16 copy + tree
                sh = sh_pool.tile([TILE_ROWS, TPR * K], FP16, tag="shd")
                nc.vector.tensor_copy(out=sh[:], in_=ps[:])
                v = sh[:].rearrange("p (g l) -> p g l", l=L)
                f3 = sh_pool.tile([TILE_ROWS, TPR * G, 8], FP16, tag="f3d")
                nc.vector.tensor_tensor(out=f3[:], in0=v[:, :, 0:8],
                                        in1=v[:, :, 8:16],
                                        op=mybir.AluOpType.max)
                f2 = sh_pool.tile([TILE_ROWS, TPR * G, 4], FP16, tag="f2d")
                nc.vector.tensor_tensor(out=f2[:], in0=f3[:, :, 0:4],
                                        in1=f3[:, :, 4:8],
                                        op=mybir.AluOpType.max)
                f1 = sh_pool.tile([TILE_ROWS, TPR * G, 2], FP16, tag="f1d")
                nc.vector.tensor_tensor(out=f1[:], in0=f2[:, :, 0:2],
                                        in1=f2[:, :, 2:4],
                                        op=mybir.AluOpType.max)
                nc.vector.tensor_tensor(
                    out=out_gm.rearrange("p (g l) -> p g l", l=1),
                    in0=f1[:, :, 0:1], in1=f1[:, :, 1:2],
                    op=mybir.AluOpType.max)
                return
            assert mode == "shadow", mode
            sh = sh_pool.tile([TILE_ROWS, TPR * K], FP16, tag="sh")
            nc.scalar.copy(sh[:], ps[:])
            v = sh[:].rearrange("p (g l) -> p g l", l=L)
            f3 = sh_pool.tile([TILE_ROWS, TPR * G, 8], FP16, tag="f3")
            nc.vector.tensor_tensor(out=f3[:], in0=v[:, :, 0:8],
                                    in1=v[:, :, 8:16],
                                    op=mybir.AluOpType.max)
            f2 = sh_pool.tile([TILE_ROWS, TPR * G, 4], FP16, tag="f2")
            nc.vector.tensor_tensor(out=f2[:], in0=f3[:, :, 0:4],
                                    in1=f3[:, :, 4:8],
                                    op=mybir.AluOpType.max)
            f1 = sh_pool.tile([TILE_ROWS, TPR * G, 2], FP16, tag="f1")
            nc.vector.tensor_tensor(out=f1[:], in0=f2[:, :, 0:2],
                                    in1=f2[:, :, 2:4],
                                    op=mybir.AluOpType.max)
            nc.vector.tensor_tensor(
                out=out_gm.rearrange("p (g l) -> p g l", l=1),
                in0=f1[:, :, 0:1], in1=f1[:, :, 1:2],
                op=mybir.AluOpType.max)

        def body():
            n_chunks = (n_tiles + CHT - 1) // CHT
            for c in range(n_chunks):
                t0 = c * CHT
                t1 = min(t0 + CHT, n_tiles)
                rows = (t1 - t0) * TILE_ROWS
                lchunk = lchunk_pool.tile([D, CHT * TILE_ROWS], F32R,
                                          tag="lchunk")
                nc.sync.dma_start(lchunk[:, :rows],
                                  lat_t[:, t0 * TILE_ROWS: t1 * TILE_ROWS])
                for p in range((t1 - t0) // TPR):
                    # TPR row-tiles share one psum tile and one grouped reduce
                    tp = t0 + TPR * p
                    ps = psum_pool.tile([TILE_ROWS, TPR * K], F32, tag="ps")
                    for r in range(TPR):
                        lt = lchunk[:, (TPR * p + r) * TILE_ROWS:
                                    (TPR * p + r + 1) * TILE_ROWS]
                        if one_mm:
                            # one matmul per row-tile: a single weight load
                            # (walrus runs with ldw-opt disabled, so each
                            # matmul instruction reloads the PE array)
                            nc.tensor.matmul(
                                ps[:, r * K:(r + 1) * K],
                                lt, c2t_sb[:],
                                start=True, stop=True)
                        else:
                            for h in range(2):
                                nc.tensor.matmul(
                                    ps[:, r * K + h * 512: r * K + (h + 1) * 512],
                                    lt, c2t_sb[:, h * 512:(h + 1) * 512],
                                    start=True, stop=True)
                    reduce_pair(ps, tp)
                # stream this chunk's group-maxes out now so the output DMA
                # overlaps later chunks instead of serializing at the tail
                if emit_out:
                    nc.sync.dma_start(gm_out[:, G * t0:G * t1],
                                      staging_gm[:, G * t0:G * t1])

        if reps == 1:
            body()
        else:
            assert reps % unroll == 0
            with tc.For_i(0, reps // unroll, 1):
                for _ in range(unroll):
                    body()

    nc.compile()
    return nc


def _get_program():
    if "nc" not in _CACHE:
        _CACHE["nc"] = _build_program()
    return _CACHE["nc"]


def kernel(latent: np.ndarray, coords: np.ndarray) -> np.ndarray:
    latent = np.asarray(latent, dtype=np.float32)
    coords = np.asarray(coords, dtype=np.float32)
    assert latent.shape == (N, D) and coords.shape == (K, D)

    nc = _get_program()

    c2_64 = (coords.astype(np.float64) ** 2).sum(1)
    order = np.argsort(c2_64, kind="stable").astype(np.int64)
    c2t = np.ascontiguousarray(2.0 * coords[order].T)

    in_maps = []
    for c in range(N_CORES):
        sl = slice(c * ROWS_PER_CORE, (c + 1) * ROWS_PER_CORE)
        in_maps.append({
            "lat_t": np.ascontiguousarray(latent[sl].T),
            "c2t": c2t,
        })

    res = run_bass_kernel_spmd(nc, in_maps, list(range(N_CORES)))

    # gm staging layout [p, G*t + g]: row n = core*ROWS + t*128 + p
    gmax = np.concatenate(
        [res.results[c]["gm"].reshape(TILE_ROWS, N_TILES, G)
         .transpose(1, 0, 2).reshape(-1, G) for c in range(N_CORES)])
    gmax = gmax.astype(np.float32)

    return _host_finish(latent, coords, gmax, c2_64, order,
                        margin=THETA + 2 * FP16_MARGIN)


def _host_finish(lat, coords, gmax_u, c2, order, n=N, margin=THETA):
    """gmax_u [n, G]: device per-group maxes of raw u = 2x.c (c2-sorted).

    Brackets each group's best score, prunes, and resolves candidates in
    fp64 with first-original-index tie-breaking.
    """
    c2s = c2[order]                               # ascending
    c2min = c2s.reshape(G, L).min(1)
    c2max = c2s.reshape(G, L).max(1)

    ub = gmax_u - c2min[None, :].astype(np.float32)
    lb = gmax_u - c2max[None, :].astype(np.float32)
    best_lb = lb.max(1)
    cand = ub >= (best_lb[:, None] - margin)      # [n, G] candidate groups

    lat64 = lat.astype(np.float64)
    coords64 = coords.astype(np.float64)
    cs64 = coords64[order].reshape(G, L, D)
    c2g = c2s.reshape(G, L)
    order_g = order.reshape(G, L)

    n_cand = cand.sum(1)
    out = np.empty(n, np.int64)

    # bulk path: rows with few candidate groups, padded to a fixed width
    CMAX = 6
    bulk = np.flatnonzero(n_cand <= CMAX)
    if bulk.size:
        # top-CMAX groups by upper bound (superset of the candidates)
        gsel = np.argpartition(-ub[bulk], CMAX - 1, axis=1)[:, :CMAX]  # [m,C]
        m = bulk.size
        cands = cs64[gsel]                        # [m, C, L, D]
        sc = 2.0 * np.einsum('md,mcld->mcl', lat64[bulk], cands,
                             optimize=True) - c2g[gsel]
        sc = sc.reshape(m, CMAX * L)
        orig = order_g[gsel].reshape(m, CMAX * L)
        # argmax with smallest-original-index tie-break
        best = sc.max(1)
        is_best = sc >= best[:, None]
        masked = np.where(is_best, orig, np.int64(1 << 60))
        out[bulk] = masked.min(1)
    rest = np.flatnonzero(n_cand > CMAX)
    if rest.size:
        sc = 2.0 * lat64[rest] @ coords64.T - c2[None, :]
        best = sc.max(1)
        is_best = sc >= best[:, None]
        masked = np.where(is_best, np.arange(K)[None, :], np.int64(1 << 60))
        out[rest] = masked.min(1)
    return out.astype(np.int32)



# revision 25
# speedup vs baseline: 1.0094x; 1.0094x over previous
"""CentroidPool (knn argmin) Trainium2 kernel.

kernel(latent [131072,128] f32, coords [1024,128] f32) -> closest-centroid
index per row, int32 [131072].

Strategy: data-parallel over rows across 8 NeuronCores. The host sorts the
1024 centroids by |c|^2 so each contiguous group of 16 has a tight |c|^2
range. Each core computes, per 128-row tile, raw scores u = 2*x@c_sorted.T
via float32r matmuls (PSUM) and reduces them to 64 per-group maxes. The
-|c|^2 term is NOT applied on device: since argmin(|x-c|^2) =
argmax(2x.c - |c|^2), the host brackets each group's best score in
[umax_g - c2max_g, umax_g - c2min_g], keeps the groups whose upper bound
reaches the best lower bound (plus a noise margin), and resolves those few
candidate groups exactly in fp64. The grouped max runs as an fp16 "shadow":
the Scalar engine converts each PSUM score block to fp16 in SBUF, and the
Vector engine folds groups with tensor_tensor max in its 2x 16-bit mode;
the fp16 rounding is absorbed into the host pruning margin.

Why this is the floor (measured on HW, see the mode zoo in reduce_pair):
the PSUM *read* port is shared across ACT and DVE at ~1 elem/cycle/lane
aggregate (~1.05 G f32/s/lane), element-rate-limited:
  - ACT-only drain (copy, no reduce): 127.0 us. Full kernel: ~124-126 us.
  - DVE PSUM reads are ~0.9 G/s/lane (2 cyc/elem) in EVERY instruction
    form: tensor_reduce (145 us all-DVE), tensor_copy (141.5 us copy-only,
    no tree), tensor_tensor. ACT is strictly the faster drain engine.
  - Any ACT/DVE drain mix: no overlap gain (128-166 us) - shared port.
  - 2-byte strided reads (bf16 view of hi-halves): no faster - element
    rate, not byte rate. Bank-interleaved APs: no faster.
  - tensor_tensor with BOTH operands in PSUM: illegal (NCC_IBVF027).
  - DMA cannot read PSUM (bass assert); GPSIMD tensor_reduce is
    partition-axis only; matmul out >512 f32 cols fails the ISA check;
    16-bit PSUM accumulate is TRN3-only.
So every one of the 1024 scores/row crosses the port exactly once, via the
fastest reader (ACT), and 16384 rows/core x 1024 / (128 lanes x 1.05G/s)
~ 122 us is the architectural floor; TensorE (75 us incl. per-matmul
weight reloads - walrus runs with ldw-opt disabled) and the DVE fold tree
(~32 us) hide underneath it. Score-packing (2 scores/f32 word), sum/
moment/LSE group bounds, and matched-pair tricks all fail on accumulator
precision or vacuous high-dimensional bounds. Device time is also
session-dependent (~ +/-5% device clock/tenancy states observed for
identical NEFFs).

Measurement note: each tc.For_i hardware-loop iteration costs a ~11 us
pipeline drain/refill barrier (measured via unroll 1/2/4/8/16: 124.6 /
119.0 / 115.7 / 114.1 / 113.2 us per pass, -0.6 more at 32). A
single-pass kernel does not pay it, so hw timing unrolls the loop body
32x (test.py); true steady-state per-pass cost is ~112.5 us against the
~110 us ACT-drain busy floor. At steady state, chunk_tiles 4 vs 8 and
sh_bufs 3 vs 4 are ties - the pipeline has no remaining stalls beyond
per-instruction PSUM-access bubbles.
"""

from contextlib import ExitStack

import numpy as np

import concourse.bacc as bacc
import concourse.mybir as mybir
import concourse.tile as tile
from concourse.bass_utils import run_bass_kernel_spmd

N = 131072
D = 128
K = 1024
N_CORES = 8
ROWS_PER_CORE = N // N_CORES        # 16384
TILE_ROWS = 128
N_TILES = ROWS_PER_CORE // TILE_ROWS  # 128
CHUNK_TILES = 4
L = 16                               # centroids per group
G = K // L                           # 64 groups
THETA = 2e-2                         # float32r noise margin for group pruning
FP16_MARGIN = 0.35                   # fp16 shadow rounding bound on |u|<=600

F32 = mybir.dt.float32
F32R = mybir.dt.float32r
FP16 = mybir.dt.float16

_CACHE: dict = {}


def _build_program(n_tiles: int = N_TILES, input_tiles: int | None = None,
                   reps: int = 1, tiles_per_reduce: int = 2,
                   psum_bufs: int = 2, chunk_tiles: int = CHUNK_TILES,
                   pattern: tuple = ("shadow",), sh_bufs: int = 3,
                   lchunk_bufs: int = 3, one_mm: bool = False,
                   unroll: int = 1):
    """pattern: per-pair reduce modes, cycled. Modes:

    - shadow:   ScalarE copies the whole PSUM pair to fp16 SBUF; VectorE
                folds groups with a 2x-mode fp16 tensor_tensor tree.
    - direct16: one VectorE grouped tensor_reduce straight from PSUM f32,
                fp16 out.
    - fold1mix: ScalarE copies the upper half of each group (l=8:16) to
                fp16 SBUF; VectorE max-folds it against the lower half
                still in PSUM (one PSUM stream - legal), then a single
                grouped reduce of the fp16 l=8 intermediate.
    """
    nc = bacc.Bacc("TRN2", target_bir_lowering=False, debug=False,
                   num_devices=N_CORES)
    n_rows = (input_tiles or n_tiles) * TILE_ROWS
    TPR = tiles_per_reduce
    CHT = chunk_tiles

    lat_t = nc.dram_tensor("lat_t", [D, n_rows], F32R, kind="ExternalInput").ap()
    c2t = nc.dram_tensor("c2t", [D, K], F32R, kind="ExternalInput").ap()
    gm_dt = FP16
    gm_out = nc.dram_tensor("gm", [TILE_ROWS, G * n_tiles], gm_dt,
                            kind="ExternalOutput").ap()

    with ExitStack() as ctx:
        tc = ctx.enter_context(tile.TileContext(nc))
        const_pool = ctx.enter_context(tc.tile_pool(name="const", bufs=1))
        stage_pool = ctx.enter_context(tc.tile_pool(name="stage", bufs=1))
        lchunk_pool = ctx.enter_context(tc.tile_pool(name="lchunk",
                                                     bufs=lchunk_bufs))
        psum_pool = ctx.enter_context(tc.tile_pool(name="psum", bufs=psum_bufs,
                                                   space="PSUM"))
        sh_pool = ctx.enter_context(tc.tile_pool(name="sh", bufs=sh_bufs))

        c2t_sb = const_pool.tile([D, K], F32R)
        nc.sync.dma_start(c2t_sb[:], c2t[:])

        # timing-diagnostic patterns never write staging; skip output then
        emit_out = any(m not in ("none", "shadow_notree", "act_bankint",
                                 "dvecopy_notree")
                       for m in pattern)
        staging_gm = None
        if emit_out:
            staging_gm = stage_pool.tile([TILE_ROWS, G * n_tiles], gm_dt,
                                         tag="staging_gm")

        assert n_tiles % TPR == 0 and CHT % TPR == 0

        def reduce_pair(ps, tp):
            mode = pattern[(tp // TPR) % len(pattern)]
            if mode == "none":      # timing diagnostic only: skip the reduce
                return
            if mode == "shadow_notree":  # diagnostic: ACT copy, no DVE work
                sh = sh_pool.tile([TILE_ROWS, TPR * K], FP16, tag="sh")
                nc.scalar.copy(sh[:], ps[:])
                return
            if mode == "dvecopy_notree":  # diagnostic: DVE PSUM copy rate
                sh = sh_pool.tile([TILE_ROWS, TPR * K], FP16, tag="shdn")
                nc.vector.tensor_copy(out=sh[:], in_=ps[:])
                return
            if mode == "act_bankint":  # diagnostic: bank-interleaved PSUM read
                sh = sh_pool.tile([TILE_ROWS, 512, 4], FP16, tag="shbi")
                nc.scalar.copy(sh[:],
                               ps[:].rearrange("p (b n) -> p n b", b=4))
                return
            out_gm = staging_gm[:, G * tp:G * (tp + TPR)]
            if mode == "shadow_r1":  # ACT copy + single grouped DVE reduce
                sh = sh_pool.tile([TILE_ROWS, TPR * K], FP16, tag="sh")
                nc.scalar.copy(sh[:], ps[:])
                nc.vector.tensor_reduce(
                    out=out_gm,
                    in_=sh[:].rearrange("p (g l) -> p g l", l=L),
                    axis=mybir.AxisListType.X, op=mybir.AluOpType.max)
                return
            if mode in ("shadow_b16", "direct16_b16"):
                # read only the high 2 bytes of each PSUM f32 (bf16
                # truncation) - halves PSUM port bytes if byte-limited
                hi = (ps[:].bitcast(mybir.dt.bfloat16)
                      .rearrange("p (n two) -> p n two", two=2)[:, :, 1:2])
                if mode == "shadow_b16":
                    sh = sh_pool.tile([TILE_ROWS, TPR * K, 1], FP16, tag="shb")
                    nc.scalar.copy(sh[:], hi)
                    v = sh[:].rearrange("p (g l) o -> p g (l o)", l=L)
                    f3 = sh_pool.tile([TILE_ROWS, TPR * G, 8], FP16, tag="f3b")
                    nc.vector.tensor_tensor(out=f3[:], in0=v[:, :, 0:8],
                                            in1=v[:, :, 8:16],
                                            op=mybir.AluOpType.max)
                    f2 = sh_pool.tile([TILE_ROWS, TPR * G, 4], FP16, tag="f2b")
                    nc.vector.tensor_tensor(out=f2[:], in0=f3[:, :, 0:4],
                                            in1=f3[:, :, 4:8],
                                            op=mybir.AluOpType.max)
                    f1 = sh_pool.tile([TILE_ROWS, TPR * G, 2], FP16, tag="f1b")
                    nc.vector.tensor_tensor(out=f1[:], in0=f2[:, :, 0:2],
                                            in1=f2[:, :, 2:4],
                                            op=mybir.AluOpType.max)
                    nc.vector.tensor_tensor(
                        out=out_gm.rearrange("p (g l) -> p g l", l=1),
                        in0=f1[:, :, 0:1], in1=f1[:, :, 1:2],
                        op=mybir.AluOpType.max)
                else:
                    vg = hi.rearrange("p (g l) two -> p g (l two)", l=L)
                    nc.vector.tensor_reduce(
                        out=out_gm,
                        in_=vg, axis=mybir.AxisListType.X,
                        op=mybir.AluOpType.max)
                return
            if mode == "direct16":
                nc.vector.tensor_reduce(
                    out=out_gm,
                    in_=ps[:].rearrange("p (g l) -> p g l", l=L),
                    axis=mybir.AxisListType.X, op=mybir.AluOpType.max)
                return
            if mode == "pool16":    # probe: GPSIMD grouped reduce from PSUM
                nc.gpsimd.tensor_reduce(
                    out=out_gm,
                    in_=ps[:].rearrange("p (g l) -> p g l", l=L),
                    axis=mybir.AxisListType.X, op=mybir.AluOpType.max)
                return
            if mode == "dma16":     # DMA drains PSUM->SBUF f32; DVE reduces
                shf = sh_pool.tile([TILE_ROWS, TPR * K], F32, tag="shf")
                nc.sync.dma_start(shf[:], ps[:])
                nc.vector.tensor_reduce(
                    out=out_gm,
                    in_=shf[:].rearrange("p (g l) -> p g l", l=L),
                    axis=mybir.AxisListType.X, op=mybir.AluOpType.max)
                return
            if mode == "pool16f":   # ACT copies f32->SBUF; GPSIMD reduces
                shf = sh_pool.tile([TILE_ROWS, TPR * K], F32, tag="shf2")
                nc.scalar.copy(shf[:], ps[:])
                nc.gpsimd.tensor_reduce(
                    out=out_gm,
                    in_=shf[:].rearrange("p (g l) -> p g l", l=L),
                    axis=mybir.AxisListType.X, op=mybir.AluOpType.max)
                return
            if mode == "fold1mix":
                v = ps[:].rearrange("p (g l) -> p g l", l=L)
                sh8 = sh_pool.tile([TILE_ROWS, TPR * G, 8], FP16, tag="sh8")
                nc.scalar.copy(sh8[:], v[:, :, 8:16])
                f3 = sh_pool.tile([TILE_ROWS, TPR * G, 8], FP16, tag="f3x")
                nc.vector.tensor_tensor(out=f3[:], in0=v[:, :, 0:8],
                                        in1=sh8[:],
                                        op=mybir.AluOpType.max)
                nc.vector.tensor_reduce(
                    out=out_gm,
                    in_=f3[:], axis=mybir.AxisListType.X,
                    op=mybir.AluOpType.max)
                return
            if mode == "dvecopy":   # DVE does the PSUM->fp16 copy + tree
                sh = sh_pool.tile([TILE_ROWS, TPR * K], FP16, tag="shd")
                nc.vector.tensor_copy(out=sh[:], in_=ps[:])
                v = sh[:].rearrange("p (g l) -> p g l", l=L)
                f3 = sh_pool.tile([TILE_ROWS, TPR * G, 8], FP16, tag="f3d")
                nc.vector.tensor_tensor(out=f3[:], in0=v[:, :, 0:8],
                                        in1=v[:, :, 8:16],
                                        op=mybir.AluOpType.max)
                f2 = sh_pool.tile([TILE_ROWS, TPR * G, 4], FP16, tag="f2d")
                nc.vector.tensor_tensor(out=f2[:], in0=f3[:, :, 0:4],
                                        in1=f3[:, :, 4:8],
                                        op=mybir.AluOpType.max)
                f1 = sh_pool.tile([TILE_ROWS, TPR * G, 2], FP16, tag="f1d")
                nc.vector.tensor_tensor(out=f1[:], in0=f2[:, :, 0:2],
                                        in1=f2[:, :, 2:4],
                                        op=mybir.AluOpType.max)
                nc.vector.tensor_tensor(
                    out=out_gm.rearrange("p (g l) -> p g l", l=1),
                    in0=f1[:, :, 0:1], in1=f1[:, :, 1:2],
                    op=mybir.AluOpType.max)
                return
            assert mode == "shadow", mode
            sh = sh_pool.tile([TILE_ROWS, TPR * K], FP16, tag="sh")
            nc.scalar.copy(sh[:], ps[:])
            v = sh[:].rearrange("p (g l) -> p g l", l=L)
            f3 = sh_pool.tile([TILE_ROWS, TPR * G, 8], FP16, tag="f3")
            nc.vector.tensor_tensor(out=f3[:], in0=v[:, :, 0:8],
                                    in1=v[:, :, 8:16],
                                    op=mybir.AluOpType.max)
            f2 = sh_pool.tile([TILE_ROWS, TPR * G, 4], FP16, tag="f2")
            nc.vector.tensor_tensor(out=f2[:], in0=f3[:, :, 0:4],
                                    in1=f3[:, :, 4:8],
                                    op=mybir.AluOpType.max)
            f1 = sh_pool.tile([TILE_ROWS, TPR * G, 2], FP16, tag="f1")
            nc.vector.tensor_tensor(out=f1[:], in0=f2[:, :, 0:2],
                                    in1=f2[:, :, 2:4],
                                    op=mybir.AluOpType.max)
            nc.vector.tensor_tensor(
                out=out_gm.rearrange("p (g l) -> p g l", l=1),
                in0=f1[:, :, 0:1], in1=f1[:, :, 1:2],
                op=mybir.AluOpType.max)

        def body():
            n_chunks = (n_tiles + CHT - 1) // CHT
            for c in range(n_chunks):
                t0 = c * CHT
                t1 = min(t0 + CHT, n_tiles)
                rows = (t1 - t0) * TILE_ROWS
                lchunk = lchunk_pool.tile([D, CHT * TILE_ROWS], F32R,
                                          tag="lchunk")
                nc.sync.dma_start(lchunk[:, :rows],
                                  lat_t[:, t0 * TILE_ROWS: t1 * TILE_ROWS])
                for p in range((t1 - t0) // TPR):
                    # TPR row-tiles share one psum tile and one grouped reduce
                    tp = t0 + TPR * p
                    ps = psum_pool.tile([TILE_ROWS, TPR * K], F32, tag="ps")
                    for r in range(TPR):
                        lt = lchunk[:, (TPR * p + r) * TILE_ROWS:
                                    (TPR * p + r + 1) * TILE_ROWS]
                        if one_mm:
                            # one matmul per row-tile: a single weight load
                            # (walrus runs with ldw-opt disabled, so each
                            # matmul instruction reloads the PE array)
                            nc.tensor.matmul(
                                ps[:, r * K:(r + 1) * K],
                                lt, c2t_sb[:],
                                start=True, stop=True)
                        else:
                            for h in range(2):
                                nc.tensor.matmul(
                                    ps[:, r * K + h * 512: r * K + (h + 1) * 512],
                                    lt, c2t_sb[:, h * 512:(h + 1) * 512],
                                    start=True, stop=True)
                    reduce_pair(ps, tp)
                # stream this chunk's group-maxes out now so the output DMA
                # overlaps later chunks instead of serializing at the tail
                if emit_out:
                    nc.sync.dma_start(gm_out[:, G * t0:G * t1],
                                      staging_gm[:, G * t0:G * t1])

        if reps == 1:
            body()
        else:
            assert reps % unroll == 0
            with tc.For_i(0, reps // unroll, 1):
                for _ in range(unroll):
                    body()

    nc.compile()
    return nc


def _get_program():
    if "nc" not in _CACHE:
        _CACHE["nc"] = _build_program()
    return _CACHE["nc"]


def kernel(latent: np.ndarray, coords: np.ndarray) -> np.ndarray:
    latent = np.asarray(latent, dtype=np.float32)
    coords = np.asarray(coords, dtype=np.float32)
    assert latent.shape == (N, D) and coords.shape == (K, D)

    nc = _get_program()

    c2_64 = (coords.astype(np.float64) ** 2).sum(1)
    order = np.argsort(c2_64, kind="stable").astype(np.int64)
    c2t = np.ascontiguousarray(2.0 * coords[order].T)

    in_maps = []
    for c in range(N_CORES):
        sl = slice(c * ROWS_PER_CORE, (c + 1) * ROWS_PER_CORE)
        in_maps.append({
            "lat_t": np.ascontiguousarray(latent[sl].T),
            "c2t": c2t,
        })

    res = run_bass_kernel_spmd(nc, in_maps, list(range(N_CORES)))

    # gm staging layout [p, G*t + g]: row n = core*ROWS + t*128 + p
    gmax = np.concatenate(
        [res.results[c]["gm"].reshape(TILE_ROWS, N_TILES, G)
         .transpose(1, 0, 2).reshape(-1, G) for c in range(N_CORES)])
    gmax = gmax.astype(np.float32)

    return _host_finish(latent, coords, gmax, c2_64, order,
                        margin=THETA + 2 * FP16_MARGIN)


def _host_finish(lat, coords, gmax_u, c2, order, n=N, margin=THETA):
    """gmax_u [n, G]: device per-group maxes of raw u = 2x.c (c2-sorted).

    Brackets each group's best score, prunes, and resolves candidates in
    fp64 with first-original-index tie-breaking.
    """
    c2s = c2[order]                               # ascending
    c2min = c2s.reshape(G, L).min(1)
    c2max = c2s.reshape(G, L).max(1)

    ub = gmax_u - c2min[None, :].astype(np.float32)
    lb = gmax_u - c2max[None, :].astype(np.float32)
    best_lb = lb.max(1)
    cand = ub >= (best_lb[:, None] - margin)      # [n, G] candidate groups

    lat64 = lat.astype(np.float64)
    coords64 = coords.astype(np.float64)
    cs64 = coords64[order].reshape(G, L, D)
    c2g = c2s.reshape(G, L)
    order_g = order.reshape(G, L)

    n_cand = cand.sum(1)
    out = np.empty(n, np.int64)

    # bulk path: rows with few candidate groups, padded to a fixed width
    CMAX = 6
    bulk = np.flatnonzero(n_cand <= CMAX)
    if bulk.size:
        # top-CMAX groups by upper bound (superset of the candidates)
        gsel = np.argpartition(-ub[bulk], CMAX - 1, axis=1)[:, :CMAX]  # [m,C]
        m = bulk.size
        cands = cs64[gsel]                        # [m, C, L, D]
        sc = 2.0 * np.einsum('md,mcld->mcl', lat64[bulk], cands,
                             optimize=True) - c2g[gsel]
        sc = sc.reshape(m, CMAX * L)
        orig = order_g[gsel].reshape(m, CMAX * L)
        # argmax with smallest-original-index tie-break
        best = sc.max(1)
        is_best = sc >= best[:, None]
        masked = np.where(is_best, orig, np.int64(1 << 60))
        out[bulk] = masked.min(1)
    rest = np.flatnonzero(n_cand > CMAX)
    if rest.size:
        sc = 2.0 * lat64[rest] @ coords64.T - c2[None, :]
        best = sc.max(1)
        is_best = sc >= best[:, None]
        masked = np.where(is_best, np.arange(K)[None, :], np.int64(1 << 60))
        out[rest] = masked.min(1)
    return out.astype(np.int32)

